# revision 3
# baseline (speedup 1.0000x reference)
"""Trainium2 Bass kernel for nn_ContrastiveLoss (NT-Xent-style loss with
tag/document masking).

Strategy v2 (8 NeuronCores, SPMD):
  - Host: L2-normalize the concatenated embeddings, cast to fp8e4m3, and
    lay the transposed reps out DoubleRow-interleaved ([128, 2, 8192]:
    (p, k, n) = z[n, k*128+p]).  Each core gets the array with columns
    ROLLED so its own 1024 rows sit at columns [0:1024] (pure SPMD).
  - Device: for each of 8 row tiles, 16 fp8 DoubleRow matmuls (K=256 in
    one instruction, 2x PE rate) fill two ping-ponged [128, 2048] PSUM
    tiles; a single wide exp activation per 2048-col group produces the
    row-sum via accum_out.  No masking, no normalization, no DVE work on
    device - the device only computes UNMASKED row sums of exp(2*sim).
  - Host: subtracts the masked terms exactly.  Masked pairs are sparse
    (same-tag ~82/row, same-doc ~16/row), so the host computes
    exp(2*z8_r.z8_c) for just those pairs via per-group GEMMs on the SAME
    fp8-rounded values the device saw (exact cancellation), and computes
    the numerator exp(2*z_r.z_partner) in full precision.
"""

import sys

for _p in ("/opt/trn_rl_repo", "/root/.axon_site/_ro/trn_rl_repo"):
    if _p not in sys.path:
        sys.path.insert(0, _p)

import ml_dtypes
import numpy as np

from concourse import bacc, mybir, tile
from concourse.bass_utils import run_bass_kernel_spmd

F32 = mybir.dt.float32
F16 = mybir.dt.float16
FP8 = mybir.dt.float8e4
FP8NP = ml_dtypes.float8_e4m3

P = 128          # SBUF partitions
B = 4096         # batch
D = 256          # embedding dim
N = 2 * B        # 8192 rows/cols of the similarity matrix
KT = D // P      # 2 contraction tiles, fused into one DoubleRow matmul
CORES = 8
ROWS_PER_CORE = N // CORES      # 1024
NI = ROWS_PER_CORE // P         # 8 row tiles per core
CH = 512                        # one matmul output (one PSUM bank of fp32)
GW = 2048                       # exp-activation width (4 banks)
NG = N // GW                    # 4 col groups per row tile
TEMP_SCALE = 2.0                # 1 / TEMPERATURE


def _build_program():
    nc = bacc.Bacc(None, target_bir_lowering=False)

    dr_d = nc.declare_dram_parameter("dr", [P, KT, N], FP8, isOutput=False)
    out_d = nc.declare_dram_parameter("out", [P, NI * NG], F32, isOutput=True)

    Exp = mybir.ActivationFunctionType.Exp
    DR = mybir.MatmulPerfMode.DoubleRow

    from contextlib import ExitStack

    with tile.TileContext(nc) as tc, ExitStack() as ctx:
        persist = ctx.enter_context(tc.tile_pool(name="persist", bufs=1))
        dr_sb = persist.tile([P, KT, N], FP8, tag="dr_sb", name="dr_sb")
        sall = persist.tile([P, NI * NG], F32, tag="sall", name="sall")
        nc.sync.dma_start(dr_sb[:], dr_d[:])

        with (
            tc.tile_pool(name="psm", bufs=2, space="PSUM") as psm,
            tc.tile_pool(name="junkp", bufs=2) as junkp,
        ):
            for i in range(NI):
                lhsT = dr_sb[:, :, i * P:(i + 1) * P]
                for g in range(NG):
                    S4 = psm.tile([P, GW], F32, tag="S4")
                    for j in range(GW // CH):
                        cs = g * GW + j * CH
                        nc.tensor.matmul(
                            S4[:, j * CH:(j + 1) * CH],
                            lhsT,
                            dr_sb[:, :, cs:cs + CH],
                            start=True, stop=True, perf_mode=DR,
                        )
                    junk = junkp.tile([P, GW], F16, tag="junk")
                    s = i * NG + g
                    nc.scalar.activation(
                        junk[:], S4[:], Exp, scale=TEMP_SCALE,
                        accum_out=sall[:, s:s + 1],
                    )

        nc.sync.dma_start(out_d[:], sall[:])

    nc.compile()
    return nc


_NC_CACHE = []


def _get_nc():
    if not _NC_CACHE:
        _NC_CACHE.append(_build_program())
    return _NC_CACHE[0]


def _host_prep(emb_i, emb_j, tags, document_ids):
    emb = np.concatenate(
        [np.asarray(emb_i), np.asarray(emb_j)], axis=0).astype(np.float64)
    z = emb / np.linalg.norm(emb, axis=1, keepdims=True)   # [N, D] exact
    z8 = z.astype(np.float32).astype(FP8NP)                # what the HW sees
    z8f = z8.astype(np.float32)

    # DoubleRow-interleaved transpose: dr[p, k, n] = z8[n, k*128 + p]
    dr = np.ascontiguousarray(z8.T.reshape(KT, P, N).transpose(1, 0, 2))

    in_maps = []
    for c in range(CORES):
        in_maps.append({"dr": np.ascontiguousarray(
            np.roll(dr, -c * ROWS_PER_CORE, axis=2))})

    # Host-side exact correction for masked-out terms.  denominator mask
    # keeps (tag_neq & doc_neq); the device sums ALL columns, so subtract
    # sum over (tag_eq | doc_eq) columns of exp(2*sim8) via
    # inclusion-exclusion over equal-tag / equal-doc / equal-both groups,
    # using the same fp8-rounded values the device multiplies.
    tags2 = np.concatenate([tags, tags]).astype(np.int64)
    docs2 = np.concatenate(
        [document_ids, document_ids]).astype(np.int64)
    corr = np.zeros(N, dtype=np.float64)
    for key, sign in ((tags2, 1.0), (docs2, 1.0),
                      (tags2 * 1024 + docs2, -1.0)):
        order = np.argsort(key, kind="stable")
        sk = key[order]
        starts = np.flatnonzero(np.r_[True, sk[1:] != sk[:-1]])
        bounds = np.r_[starts, len(sk)]
        for a, b in zip(bounds[:-1], bounds[1:]):
            idx = order[a:b]
            G = z8f[idx] @ z8f[idx].T
            corr[idx] += sign * np.exp(2.0 * G.astype(np.float64)).sum(1)

    # Exact numerator exponent: log(num_r) = 2 * z_r . z_partner
    zd = (z[:B] * z[B:]).sum(1)
    zdot = np.concatenate([zd, zd])
    return in_maps, corr, zdot


def _assemble_loss(results, corr, zdot):
    rowsum = np.empty(N, dtype=np.float64)
    for c in range(CORES):
        o = np.asarray(results[c]["out"]).astype(np.float64)
        # o[p, i*NG + g] = partial row sum (cols g*2048..) of local row i*128+p
        per_row = o.reshape(P, NI, NG).sum(2)              # [128, 8]
        r0 = c * ROWS_PER_CORE
        rowsum[r0:r0 + ROWS_PER_CORE] = per_row.T.reshape(-1)
    denom = rowsum - corr + 0.1
    loss = (np.log(denom) - TEMP_SCALE * zdot).sum() / N
    return np.float32(loss)


def kernel(emb_i, emb_j, tags, num_classes, document_ids):
    nc = _get_nc()
    in_maps, corr, zdot = _host_prep(emb_i, emb_j, tags, document_ids)
    res = run_bass_kernel_spmd(nc, in_maps, list(range(CORES)))
    return _assemble_loss(res.results, corr, zdot)


# revision 5
# speedup vs baseline: 1.3520x; 1.3520x over previous
"""Trainium2 Bass kernel for nn_ContrastiveLoss (NT-Xent-style loss with
tag/document masking).

Strategy v2 (8 NeuronCores, SPMD):
  - Host: L2-normalize the concatenated embeddings, cast to fp8e4m3, and
    lay the transposed reps out DoubleRow-interleaved ([128, 2, 8192]:
    (p, k, n) = z[n, k*128+p]).  Each core gets the array with columns
    ROLLED so its own 1024 rows sit at columns [0:1024] (pure SPMD).
  - Device: for each of 8 row tiles, 16 fp8 DoubleRow matmuls (K=256 in
    one instruction, 2x PE rate) fill two ping-ponged [128, 2048] PSUM
    tiles; a single wide exp activation per 2048-col group produces the
    row-sum via accum_out.  No masking, no normalization, no DVE work on
    device - the device only computes UNMASKED row sums of exp(2*sim).
  - Host: subtracts the masked terms exactly.  Masked pairs are sparse
    (same-tag ~82/row, same-doc ~16/row), so the host computes
    exp(2*z8_r.z8_c) for just those pairs via per-group GEMMs on the SAME
    fp8-rounded values the device saw (exact cancellation), and computes
    the numerator exp(2*z_r.z_partner) in full precision.
"""

import sys

for _p in ("/opt/trn_rl_repo", "/root/.axon_site/_ro/trn_rl_repo"):
    if _p not in sys.path:
        sys.path.insert(0, _p)

import ml_dtypes
import numpy as np

from concourse import bacc, mybir, tile
from concourse.bass_utils import run_bass_kernel_spmd

F32 = mybir.dt.float32
F16 = mybir.dt.float16
FP8 = mybir.dt.float8e4
FP8NP = ml_dtypes.float8_e4m3

P = 128          # SBUF partitions
B = 4096         # batch
D = 256          # embedding dim
N = 2 * B        # 8192 rows/cols of the similarity matrix
KT = D // P      # 2 contraction tiles, fused into one DoubleRow matmul
CORES = 8
ROWS_PER_CORE = N // CORES      # 1024
NI = ROWS_PER_CORE // P         # 8 row tiles per core
CH = 512                        # one matmul output (one PSUM bank of fp32)
GW = 2048                       # exp-activation width (4 banks)
NG = N // GW                    # 4 col groups per row tile
TEMP_SCALE = 2.0                # 1 / TEMPERATURE


def _build_program():
    nc = bacc.Bacc(None, target_bir_lowering=False)

    lhs_d = nc.declare_dram_parameter("lhs", [P, KT, ROWS_PER_CORE], FP8,
                                      isOutput=False)
    dr_d = [nc.declare_dram_parameter(f"dr{q}", [P, KT, GW], FP8,
                                      isOutput=False) for q in range(NG)]
    out_d = nc.declare_dram_parameter("out", [P, NI * NG], F32, isOutput=True)

    Exp = mybir.ActivationFunctionType.Exp
    DR = mybir.MatmulPerfMode.DoubleRow

    from contextlib import ExitStack

    with tile.TileContext(nc) as tc, ExitStack() as ctx:
        persist = ctx.enter_context(tc.tile_pool(name="persist", bufs=1))
        lhs_sb = persist.tile([P, KT, ROWS_PER_CORE], FP8, tag="lhs_sb",
                              name="lhs_sb")
        dr_sb = [persist.tile([P, KT, GW], FP8, tag=f"dr_sb{q}",
                              name=f"dr_sb{q}") for q in range(NG)]
        sall = persist.tile([P, NI * NG], F32, tag="sall", name="sall")
        nc.sync.dma_start(lhs_sb[:], lhs_d[:])
        for q in range(NG):
            nc.sync.dma_start(dr_sb[q][:], dr_d[q][:])

        with (
            tc.tile_pool(name="psm", bufs=2, space="PSUM") as psm,
            tc.tile_pool(name="junkp", bufs=2) as junkp,
        ):
            for g in range(NG):
                for i in range(NI):
                    lhsT = lhs_sb[:, :, i * P:(i + 1) * P]
                    S4 = psm.tile([P, GW], F32, tag="S4")
                    for j in range(GW // CH):
                        nc.tensor.matmul(
                            S4[:, j * CH:(j + 1) * CH],
                            lhsT,
                            dr_sb[g][:, :, j * CH:(j + 1) * CH],
                            start=True, stop=True, perf_mode=DR,
                        )
                    junk = junkp.tile([P, GW], F16, tag="junk")
                    s = i * NG + g
                    nc.scalar.activation(
                        junk[:], S4[:], Exp, scale=TEMP_SCALE,
                        accum_out=sall[:, s:s + 1],
                    )

        nc.sync.dma_start(out_d[:], sall[:])

    nc.compile()
    return nc


_NC_CACHE = []


def _get_nc():
    if not _NC_CACHE:
        _NC_CACHE.append(_build_program())
    return _NC_CACHE[0]


def _host_prep(emb_i, emb_j, tags, document_ids):
    emb = np.concatenate(
        [np.asarray(emb_i), np.asarray(emb_j)], axis=0).astype(np.float64)
    z = emb / np.linalg.norm(emb, axis=1, keepdims=True)   # [N, D] exact
    z8 = z.astype(np.float32).astype(FP8NP)                # what the HW sees
    z8f = z8.astype(np.float32)

    # DoubleRow-interleaved transpose: dr[p, k, n] = z8[n, k*128 + p]
    dr = np.ascontiguousarray(z8.T.reshape(KT, P, N).transpose(1, 0, 2))

    slabs = {f"dr{q}": np.ascontiguousarray(dr[:, :, q * GW:(q + 1) * GW])
             for q in range(NG)}
    in_maps = []
    for c in range(CORES):
        r0 = c * ROWS_PER_CORE
        in_maps.append({
            "lhs": np.ascontiguousarray(dr[:, :, r0:r0 + ROWS_PER_CORE]),
            **slabs,
        })

    # Host-side exact correction for masked-out terms.  denominator mask
    # keeps (tag_neq & doc_neq); the device sums ALL columns, so subtract
    # sum over (tag_eq | doc_eq) columns of exp(2*sim8) via
    # inclusion-exclusion over equal-tag / equal-doc / equal-both groups,
    # using the same fp8-rounded values the device multiplies.
    tags2 = np.concatenate([tags, tags]).astype(np.int64)
    docs2 = np.concatenate(
        [document_ids, document_ids]).astype(np.int64)
    corr = np.zeros(N, dtype=np.float64)
    for key, sign in ((tags2, 1.0), (docs2, 1.0),
                      (tags2 * 1024 + docs2, -1.0)):
        order = np.argsort(key, kind="stable")
        sk = key[order]
        starts = np.flatnonzero(np.r_[True, sk[1:] != sk[:-1]])
        bounds = np.r_[starts, len(sk)]
        for a, b in zip(bounds[:-1], bounds[1:]):
            idx = order[a:b]
            G = z8f[idx] @ z8f[idx].T
            corr[idx] += sign * np.exp(2.0 * G.astype(np.float64)).sum(1)

    # Exact numerator exponent: log(num_r) = 2 * z_r . z_partner
    zd = (z[:B] * z[B:]).sum(1)
    zdot = np.concatenate([zd, zd])
    return in_maps, corr, zdot


def _assemble_loss(results, corr, zdot):
    rowsum = np.empty(N, dtype=np.float64)
    for c in range(CORES):
        o = np.asarray(results[c]["out"]).astype(np.float64)
        # o[p, i*NG + g] = partial row sum (cols g*2048..) of local row i*128+p
        per_row = o.reshape(P, NI, NG).sum(2)              # [128, 8]
        r0 = c * ROWS_PER_CORE
        rowsum[r0:r0 + ROWS_PER_CORE] = per_row.T.reshape(-1)
    denom = rowsum - corr + 0.1
    loss = (np.log(denom) - TEMP_SCALE * zdot).sum() / N
    return np.float32(loss)


def kernel(emb_i, emb_j, tags, num_classes, document_ids):
    nc = _get_nc()
    in_maps, corr, zdot = _host_prep(emb_i, emb_j, tags, document_ids)
    res = run_bass_kernel_spmd(nc, in_maps, list(range(CORES)))
    return _assemble_loss(res.results, corr, zdot)
